# revision 1
# baseline (speedup 1.0000x reference)
# Depthwise causal conv2d (N=2, C=16, H=W=2048, kernel 6x11) on 8 TRN2 cores.
#
# y[b,c,p,q] = sum_{r,s} w[c,r,s] * xm[b,c, p+r-5, q+s-5], xm = tril-masked x,
# y tril-masked.  Sharding: the 32 (b,c) images are independent; 4 per core.
#
# Per-core compute: for each output tile of [M<=123 rows, Nd<=512 cols], the
# row-tap contraction is a banded-Toeplitz matmul: for each of the S=11
# column taps, out[m, n] += band_s[k, m] * x[k, n+s] where band_s[k, m] =
# w[c, k-m, s] (k-m in [0,6)).  11 accumulating matmuls per tile into one
# PSUM bank.  Tiles fully above the causal diagonal are never computed or
# written (output DRAM is pre-zeroed); tiles crossing it get the causal mask
# applied to the input (gpsimd affine_select in SBUF) and to the output
# (DVE multiply with a staircase 0/1 tile during PSUM evacuation).
import sys

sys.path.insert(0, "/opt/trn_rl_repo")

import numpy as np

import concourse.bacc as bacc
import concourse.mybir as mybir
import concourse.tile as tile
from concourse.bass_utils import run_bass_kernel_spmd

N, C, H, W = 2, 16, 2048, 2048
R, S, PH, PW = 6, 11, 5, 5
NCORES = 8
IPC = (N * C) // NCORES  # images per core
MT = 123  # output rows per row-tile (128 input rows incl. 5-row halo)
NTS = 512  # max output cols per tile (one PSUM bank of fp32)
BANDW = 128  # allocated band width (cols used: M)
STAIR_W = 1536  # staircase mask width
STAIR_C = 511  # staircase offset: stair[i, u] = 1 iff u <= i + STAIR_C
F32 = mybir.dt.float32

# Matmul input dtype. Measured per-core kernel time / rel err vs fp32 ref:
#   "bf16" 392 us / 2.3e-3,  "f16" 474 us / 3.1e-4,  "f32r" 568 us / 2.5e-4.
DTYPE_MODE = "bf16"

_NC_CACHE = {}


def _xdt():
    return {
        "f16": mybir.dt.float16,
        "bf16": mybir.dt.bfloat16,
        "f32r": mybir.dt.float32r,
    }[DTYPE_MODE]


def _np_xdt():
    if DTYPE_MODE == "f16":
        return np.dtype(np.float16)
    if DTYPE_MODE == "bf16":
        import ml_dtypes

        return np.dtype(ml_dtypes.bfloat16)
    return np.dtype(np.float32)


def _row_tiles():
    out = []
    p0 = 0
    while p0 < H:
        out.append((p0, min(MT, H - p0)))
        p0 += MT
    return out


def _col_tiles(pmax):
    """Column tiles covering q in [0, pmax]; width 512 except the last,
    which shrinks to a multiple of 128 (fp32r needs >=256 for full rate)."""
    min_nd = 256 if DTYPE_MODE == "f32r" else 128
    out = []
    q0 = 0
    while q0 <= pmax:
        needed = pmax - q0 + 1
        if needed >= NTS:
            nd = NTS
        else:
            nd = min(NTS, max(min_nd, 128 * ((needed + 127) // 128)))
        out.append((q0, nd))
        q0 += nd
    return out


def _build_program(rep=1):
    """One SPMD program: conv of IPC images [H, W] with per-image bands.

    rep > 1 wraps the whole body in a hardware loop executing it `rep`
    times — benchmarking only (amplifies kernel time above the fixed
    dispatch overhead of the execution path)."""
    import contextlib

    xdt = _xdt()
    nc = bacc.Bacc("TRN2", target_bir_lowering=False, debug=False,
                   num_devices=NCORES)
    x = nc.dram_tensor("x", [IPC, H, W], xdt, kind="ExternalInput")
    bands = nc.dram_tensor("bands", [IPC, 128, S * BANDW], xdt,
                           kind="ExternalInput")
    y = nc.dram_tensor("y", [IPC, H, W], F32, kind="ExternalOutput")

    row_tiles = _row_tiles()

    with tile.TileContext(nc) as tc:
        with (
            tc.tile_pool(name="const", bufs=1) as cpool,
            tc.tile_pool(name="xin", bufs=6) as xpool,
            tc.tile_pool(name="out", bufs=4) as opool,
            tc.tile_pool(name="psum", bufs=8, space="PSUM") as ppool,
            tc.For_i(0, rep, 1) if rep > 1 else contextlib.nullcontext(),
        ):
            # Per-image Toeplitz bands, resident for the whole kernel.
            bt = cpool.tile([128, IPC * S * BANDW], xdt)
            for i in range(IPC):
                nc.sync.dma_start(
                    out=bt[:, i * S * BANDW:(i + 1) * S * BANDW],
                    in_=bands[i],
                )
            # Staircase causal mask: stair[i, u] = 1 iff i + STAIR_C - u >= 0.
            stair = cpool.tile([128, STAIR_W], F32)
            nc.gpsimd.memset(stair[:], 1.0)
            nc.gpsimd.affine_select(
                out=stair[:], in_=stair[:],
                compare_op=mybir.AluOpType.is_ge, fill=0.0,
                base=STAIR_C, channel_multiplier=1,
                pattern=[[-1, STAIR_W]],
            )

            for i in range(IPC):
                band_i = bt[:, i * S * BANDW:(i + 1) * S * BANDW]
                for (p0, M) in row_tiles:
                    pmax = p0 + M - 1
                    for (q0, nd) in _col_tiles(pmax):
                        _emit_tile(nc, tc, xpool, opool, ppool, x, y, band_i,
                                   stair, i, p0, M, pmax, q0, nd)
    nc.compile()
    return nc


def _emit_tile(nc, tc, xpool, opool, ppool, x, y, band_i, stair,
               i, p0, M, pmax, q0, nd):
    xdt = _xdt()
    inw = nd + S - 1  # input tile width incl. halo
    # Input rows [p0-PH, p0+M), cols [q0-PW, q0+nd+PW) clipped to the image
    # and (on the right) to the causal extent pmax.
    h0 = p0 - PH
    hv0, hv1 = max(0, h0), min(H, p0 + M)
    w0 = q0 - PW
    wv0 = max(0, w0)
    wv1 = min(W, q0 + nd + PW, pmax + 1)

    d = p0 - q0  # diagonal offset of this tile
    # Causal mask on input needed iff the loaded region touches h < w.
    in_mask = h0 < wv1 - 1
    # Causal mask on output needed iff the tile crosses p < q.
    out_mask = p0 < q0 + nd - 1

    xt = xpool.tile([128, inw], xdt, tag="xin")
    nc.sync.dma_start(
        out=xt[hv0 - h0:hv1 - h0, wv0 - w0:wv1 - w0],
        in_=x[i, hv0:hv1, wv0:wv1],
    )
    if q0 == 0:
        # w in [-5, 0) is zero padding that the causal select keeps
        # (memset can't write fp32r): keep iff j >= PW.
        nc.gpsimd.affine_select(
            out=xt[:], in_=xt[:],
            compare_op=mybir.AluOpType.is_ge, fill=0.0,
            base=-PW, channel_multiplier=0,
            pattern=[[1, inw]],
        )
    if in_mask:
        # keep iff (h0 + k) >= (w0 + j)  <=>  k - j + (h0 - w0) >= 0.
        # Also zeroes the unloaded right-of-pmax region and, for p0 == 0,
        # the k < PH halo rows above the image (h < 0 keeps nothing).
        nc.gpsimd.affine_select(
            out=xt[:], in_=xt[:],
            compare_op=mybir.AluOpType.is_ge, fill=0.0,
            base=h0 - w0, channel_multiplier=1,
            pattern=[[-1, inw]],
        )

    pt = ppool.tile([M, NTS], F32, tag="psum")
    for s in range(S):
        nc.tensor.matmul(
            pt[:, :nd],
            lhsT=band_i[:, s * BANDW:s * BANDW + M],
            rhs=xt[:, s:s + nd],
            start=(s == 0), stop=(s == S - 1),
        )

    # Columns q > pmax are entirely above the diagonal: skip them.
    wn = min(nd, pmax - q0 + 1)
    ot = opool.tile([128, NTS], F32, tag="out")
    if out_mask:
        # Evacuate PSUM through the causal staircase: keep iff
        # (p0+m) >= (q0+n) <=> stair[m, n + STAIR_C - d] with d = p0-q0.
        u0 = STAIR_C - d
        nc.vector.tensor_mul(
            ot[:M, :wn], pt[:M, :wn], stair[:M, u0:u0 + wn],
        )
    else:
        nc.any.tensor_copy(ot[:M, :wn], pt[:M, :wn])
    nc.sync.dma_start(
        out=y[i, p0:p0 + M, q0:q0 + wn],
        in_=ot[:M, :wn],
    )


def _build_bands(weight):
    """Host-side: per-image banded Toeplitz weights.
    bands[img, k, s*BANDW + m] = w[c(img), k-m, s] for k-m in [0, R)."""
    nimg = N * C
    bands = np.zeros((nimg, 128, S * BANDW), np.float32)
    m = np.arange(BANDW)
    for s in range(S):
        for r in range(R):
            # band[m+r, s*BANDW+m] = w[c, r, s]
            valid = m + r < 128
            mv = m[valid]
            for img in range(nimg):
                c = img % C
                bands[img, mv + r, s * BANDW + mv] = weight[c, r, s]
    return bands.astype(_np_xdt())


def kernel(x, weight):
    x = np.asarray(x, dtype=np.float32)
    weight = np.asarray(weight, dtype=np.float32)
    assert x.shape == (N, C, H, W) and weight.shape == (C, R, S)

    if "nc" not in _NC_CACHE:
        _NC_CACHE["nc"] = _build_program()
    nc = _NC_CACHE["nc"]

    x_imgs = np.ascontiguousarray(x.reshape(N * C, H, W)).astype(
        _np_xdt(), copy=False)
    bands = _build_bands(weight)
    in_maps = [
        {
            "x": x_imgs[k * IPC:(k + 1) * IPC],
            "bands": bands[k * IPC:(k + 1) * IPC],
        }
        for k in range(NCORES)
    ]
    res = run_bass_kernel_spmd(nc, in_maps, list(range(NCORES)))
    out = np.concatenate([res.results[k]["y"] for k in range(NCORES)], axis=0)
    return out.reshape(N, C, H, W)



# revision 3
# speedup vs baseline: 3.3177x; 3.3177x over previous
# Depthwise causal conv2d (N=2, C=16, H=W=2048, kernel 6x11) on 8 TRN2 cores.
#
# y[b,c,p,q] = sum_{r,s} w[c,r,s] * xm[b,c, p+r-5, q+s-5], xm = tril-masked x,
# y tril-masked.  Sharding: the 32 (b,c) images are independent; 4 per core.
#
# Column-banded transposed formulation: both x and y live TRANSPOSED in DRAM
# (xT[u=w, v=h], yT[q, p]; host transposes outside the kernel launch).  Then
#   yT[q, p] = sum_r sum_k B_r[k, q-q0] * xT[q0-5+k, p+r-5]
# with B_r[k, m] = w[c, r, k-m] (band-11 Toeplitz over the S taps).  Per
# output tile [QM=118 q x 512 p] that is SIX accumulating matmuls (one per
# row tap r; stationary = B_r, moving = a column-shifted slice of the xT
# strip) instead of the eleven the row-banded form needs: the 11-wide S band
# packs the contraction denser than the 6-wide R band.
#
# DMA: one load per (image, q0-strip) [128, <=2053] and one store per strip
# (outputs staged in SBUF as bf16), issued alternately on the SP and
# Activation HWDGE rings - ~2.5x fewer descriptors than per-tile DMA and two
# sequencers instead of one.  yT is written bf16 (halves store bytes); the
# host widens to fp32.  Causal masks: gpsimd affine_select on the input
# strip (keep v >= u), DVE staircase multiply on PSUM evacuation.
import sys

sys.path.insert(0, "/opt/trn_rl_repo")

import numpy as np

import concourse.bacc as bacc
import concourse.mybir as mybir
import concourse.tile as tile
from concourse.bass_utils import run_bass_kernel_spmd

N, C, H, W = 2, 16, 2048, 2048
R, S, PH, PW = 6, 11, 5, 5
NCORES = 8
IPC = (N * C) // NCORES  # images per core
QM = 118      # output cols (q) per strip: 128-partition contraction window
PN = 512      # output rows (p) per tile (one PSUM bank of fp32)
STW = 1152    # staircase width; stair2[i, t] = 1 iff t >= i + 512
F32 = mybir.dt.float32
BF16 = mybir.dt.bfloat16

_NC_CACHE = {}


def _np_bf16():
    import ml_dtypes

    return np.dtype(ml_dtypes.bfloat16)


def _strips():
    """(q0, qm) col-strips covering q in [0, H)."""
    out = []
    q0 = 0
    while q0 < H:
        out.append((q0, min(QM, H - q0)))
        q0 += QM
    return out


def _p_tiles(q0):
    """Row tiles (p0) with any causal output (p >= q0 somewhere)."""
    return [p0 for p0 in range(0, H, PN) if p0 + PN - 1 >= q0]


def _build_program(rep=1):
    """One SPMD program: conv of IPC transposed images with per-image bands.

    rep > 1 wraps the body in a hardware loop (benchmarking only)."""
    import contextlib

    nc = bacc.Bacc("TRN2", target_bir_lowering=False, debug=False,
                   num_devices=NCORES)
    xT = nc.dram_tensor("xT", [IPC, W, H], BF16, kind="ExternalInput")
    bands = nc.dram_tensor("bands", [IPC, 128, R * 128], BF16,
                           kind="ExternalInput")
    yT = nc.dram_tensor("yT", [IPC, W, H], BF16, kind="ExternalOutput")

    with tile.TileContext(nc) as tc:
        with (
            tc.tile_pool(name="const", bufs=1) as cpool,
            tc.tile_pool(name="xin", bufs=3) as xpool,
            tc.tile_pool(name="out", bufs=3) as opool,
            tc.tile_pool(name="psum", bufs=8, space="PSUM") as ppool,
            tc.For_i(0, rep, 1) if rep > 1 else contextlib.nullcontext(),
        ):
            # Per-image bands, resident for the whole kernel.
            bt = cpool.tile([128, IPC * R * 128], BF16)
            for i in range(IPC):
                nc.sync.dma_start(
                    out=bt[:, i * R * 128:(i + 1) * R * 128], in_=bands[i],
                )
            # Staircase mask: stair2[i, t] = 1 iff t - i - 512 >= 0.
            stair2 = cpool.tile([128, STW], F32)
            nc.gpsimd.memset(stair2[:], 1.0)
            nc.gpsimd.affine_select(
                out=stair2[:], in_=stair2[:],
                compare_op=mybir.AluOpType.is_ge, fill=0.0,
                base=-512, channel_multiplier=-1,
                pattern=[[1, STW]],
            )

            ndma = [0]

            def dma(out, in_):
                eng = nc.sync if ndma[0] % 2 == 0 else nc.scalar
                ndma[0] += 1
                eng.dma_start(out=out, in_=in_)

            for i in range(IPC):
                band_i = bt[:, i * R * 128:(i + 1) * R * 128]
                for (q0, qm) in _strips():
                    _emit_strip(nc, tc, xpool, opool, ppool, xT, yT,
                                band_i, stair2, dma, i, q0, qm)
    nc.compile()
    return nc


def _emit_strip(nc, tc, xpool, opool, ppool, xT, yT, band_i, stair2, dma,
                i, q0, qm):
    p_tiles = _p_tiles(q0)
    ps0 = p_tiles[0]
    v0 = ps0 - PH            # strip col range [v0, H) in v = p coords
    ext = H - v0             # SBUF strip width (incl. 5-col lead halo)
    u0 = q0 - PH             # partition k = u - u0, u = input col
    uv0, uv1 = max(0, u0), min(W, u0 + 128)  # valid u rows to load

    xt = xpool.tile([128, ext], BF16, tag="xin")
    # Load valid region; left halo v<0 (only when ps0==0) and u<0 / u>=W
    # partitions are zeroed by the selects below.
    lv0 = max(0, v0)
    dma(xt[uv0 - u0:uv1 - u0, lv0 - v0:], xT[i, uv0:uv1, lv0:H])

    if u0 < 0:
        # q0 == 0: partitions k < 5 are u < 0 (left w-padding): zero them.
        nc.gpsimd.memset(xt[:PH, :], 0.0)
    if v0 < 0:
        # ps0 == 0: cols j < 5 are v < 0 (top h-padding): keep iff j >= 5.
        nc.gpsimd.affine_select(
            out=xt[:, :16], in_=xt[:, :16],
            compare_op=mybir.AluOpType.is_ge, fill=0.0,
            base=-PH, channel_multiplier=0,
            pattern=[[1, 16]],
        )
    # Causal mask: keep iff v >= u <=> (v0+j) - (q0-5+k) >= 0.  Only the
    # region j <= (q0+122-v0) is ambiguous, except the last strip where
    # u >= W garbage rows must die too: extend to the full strip there.
    cw = min(ext, q0 + 123 - v0) if u0 + 128 <= W else ext
    if cw > 0:
        nc.gpsimd.affine_select(
            out=xt[:, :cw], in_=xt[:, :cw],
            compare_op=mybir.AluOpType.is_ge, fill=0.0,
            base=v0 - q0 + PH, channel_multiplier=-1,
            pattern=[[1, cw]],
        )

    ys = opool.tile([128, ext], BF16, tag="out")
    for p0 in p_tiles:
        pt = ppool.tile([128, PN], F32, tag="psum")
        for r in range(R):
            # moving operand: xT strip cols v = p0-5+r .. +PN
            j0 = p0 - PH + r - v0
            nc.tensor.matmul(
                pt[:, :],
                lhsT=band_i[:, r * 128:(r + 1) * 128],
                rhs=xt[:, j0:j0 + PN],
                start=(r == 0), stop=(r == R - 1),
            )
        # Evacuate cols p >= q0 only (left of that is all-masked).
        n0 = max(0, q0 - p0)
        d = p0 - q0
        crossing = p0 + n0 < q0 + qm - 1
        if crossing:
            nc.vector.tensor_mul(
                ys[:qm, p0 + n0 - v0:p0 + PN - v0],
                pt[:qm, n0:PN],
                stair2[:qm, 512 + d + n0:512 + d + PN],
            )
        else:
            nc.any.tensor_copy(
                ys[:qm, p0 + n0 - v0:p0 + PN - v0], pt[:qm, n0:PN],
            )
    # One store per strip: cols p in [q0, H).
    dma(yT[i, q0:q0 + qm, q0:H], ys[:qm, q0 - v0:])


def _build_bands(weight):
    """Host-side: per-image column-banded Toeplitz weights.
    bands[img, k, r*128 + m] = w[c(img), r, k-m] for k-m in [0, S)."""
    nimg = N * C
    bands = np.zeros((nimg, 128, R * 128), np.float32)
    m = np.arange(128)
    for r in range(R):
        for s in range(S):
            valid = m + s < 128
            mv = m[valid]
            for img in range(nimg):
                c = img % C
                bands[img, mv + s, r * 128 + mv] = weight[c, r, s]
    return bands.astype(_np_bf16())


def kernel(x, weight):
    x = np.asarray(x, dtype=np.float32)
    weight = np.asarray(weight, dtype=np.float32)
    assert x.shape == (N, C, H, W) and weight.shape == (C, R, S)

    if "nc" not in _NC_CACHE:
        _NC_CACHE["nc"] = _build_program()
    nc = _NC_CACHE["nc"]

    xT_imgs = np.ascontiguousarray(
        x.reshape(N * C, H, W).swapaxes(1, 2)).astype(_np_bf16(), copy=False)
    bands = _build_bands(weight)
    in_maps = [
        {
            "xT": xT_imgs[k * IPC:(k + 1) * IPC],
            "bands": bands[k * IPC:(k + 1) * IPC],
        }
        for k in range(NCORES)
    ]
    res = run_bass_kernel_spmd(nc, in_maps, list(range(NCORES)))
    yT = np.concatenate([res.results[k]["yT"] for k in range(NCORES)], axis=0)
    y = yT.astype(np.float32).swapaxes(1, 2)
    return np.ascontiguousarray(y).reshape(N, C, H, W)
